# revision 1
# baseline (speedup 1.0000x reference)
"""Bass/Trainium2 kernel for nn_BiSDA_37160057045272.

The reference module is a spiking (LIF) sparse-attention block.  Its final
stage is ``out = lif(attn_spike * v_spike)`` followed by a projection +
BatchNorm.  Both ``attn_spike`` and ``v_spike`` are Heaviside spikes in
{0, 1}, so the final LIF's input x is in [0, 1].  With the LIF update
``v <- v + (x - v)/tau`` (tau = 2, v0 = 0), the membrane potential after
T = 4 steps is bounded by 0.5 + 0.25 + 0.125 + 0.0625 = 0.9375 < V_TH = 1.0,
so the final LIF can NEVER fire, for ANY input values.  The last lif()
output is identically zero, the projection of zeros is zero, and
BatchNorm3d of a constant-zero tensor is ``0 * gamma + beta = beta``.

Hence the module computes, exactly, for every input:

    output[t, b, c, l, h, w] = p_beta[c]

(verified bit-exact against the jax reference for the spec inputs, for
random gammas/betas, and for 100x-scaled activations).

The kernel therefore broadcasts p_beta into the full output shape.  Each of
the 8 NeuronCores materializes 1/8 of the output (2 of the 16 T*B items,
i.e. a [2, 256, 8192] f32 shard = 16.8 MB) in device DRAM: p_beta is DMA'd
to SBUF, replicated across the free dimension on the vector engine, and
written out with large (multi-MB) DMAs that stripe across all 16 SDMA
engines.  The host concatenates the 8 shards into the full output.
"""

import numpy as np

import concourse.bacc as bacc
import concourse.mybir as mybir
import concourse.tile as tile
from concourse.bass_utils import run_bass_kernel_spmd


def _ensure_axon_hooks_importable():
    """Compat shim: ``bass_utils`` does a bare ``from antenv.axon_hooks
    import get_axon_ntff_profile_hook`` whenever tracing is requested
    (e.g. env BASS_TRACE=1).  This image's ``antenv`` lacks that module,
    which would turn a trace request into an ImportError.  If it is
    missing, register an equivalent module: the same ctypes NTFF-profile
    protocol against libaxon_pjrt.so that trn_boot.py uses, degrading to
    a no-hook (tracing skipped, run still works) if the .so is absent.
    """
    try:
        import antenv.axon_hooks  # noqa: F401
        return
    except ImportError:
        pass
    import contextlib
    import ctypes
    import sys
    import types

    def _make_hook():
        try:
            lib = ctypes.CDLL("/opt/axon/libaxon_pjrt.so")
            if not hasattr(lib, "axon_start_nrt_profile"):
                return None
        except OSError:
            return None
        lib.axon_start_nrt_profile.argtypes = [
            ctypes.POINTER(ctypes.c_int64),
            ctypes.c_size_t,
        ]
        lib.axon_start_nrt_profile.restype = ctypes.c_int64
        lib.axon_stop_nrt_profile.argtypes = [ctypes.c_char_p]
        lib.axon_stop_nrt_profile.restype = ctypes.c_int64

        @contextlib.contextmanager
        def _hook(output_dir, device_ids):
            import jax

            jax.devices()
            if device_ids:
                ids = (ctypes.c_int64 * len(device_ids))(*device_ids)
                rc = lib.axon_start_nrt_profile(ids, len(device_ids))
            else:
                rc = lib.axon_start_nrt_profile(None, 0)
            if rc != 0:
                raise RuntimeError(f"axon_start_nrt_profile rc={rc}")
            try:
                yield
            finally:
                lib.axon_stop_nrt_profile(str(output_dir).encode())

        return _hook

    mod = types.ModuleType("antenv.axon_hooks")
    _the_hook = _make_hook()
    mod.get_axon_ntff_profile_hook = lambda: _the_hook
    mod.set_axon_ntff_profile_hook = lambda h: None
    sys.modules["antenv.axon_hooks"] = mod


_ensure_axon_hooks_importable()

# Problem shapes (hardcoded per contract -- kernel.py must be self-contained).
T, B, C, Lt, Lh, Lw = 4, 4, 256, 8, 32, 32
N = Lt * Lh * Lw            # 8192 spatial positions
ITEMS = T * B               # 16 flattened (t, b) items
N_CORES = 8
IPC = ITEMS // N_CORES      # 2 items per core
P = 128                     # SBUF partitions
CT = C // P                 # 2 channel tiles
FILL_CHUNK = 4096           # free-dim elements per SBUF fill instruction
DMA_CHUNK = 4096            # free-dim elements per output DMA (2 MB each)
EARLY_SPANS = (512, 1024, 2048)   # leading spans so the first DMAs start early
BETA_ENGINE = "sync"        # engine issuing the p_beta load DMA
BETA_SPLIT = False          # load each beta column with its own DMA
RAMP_ALT_RING = False       # issue ramp DMAs alternately on the ACT HWDGE ring

_CACHE: dict = {}
LAST_RESULTS = None         # BassKernelResults of the last run (for test harness)


def _build_nc():
    nc = bacc.Bacc("TRN2", target_bir_lowering=False, debug=False)
    p_beta = nc.dram_tensor("p_beta", (C,), mybir.dt.float32, kind="ExternalInput")
    out = nc.dram_tensor(
        "out", (IPC, C, N), mybir.dt.float32, kind="ExternalOutput"
    )
    out_ap = out.ap()
    with tile.TileContext(nc) as tc:
        with (
            tc.tile_pool(name="beta", bufs=1) as bpool,
            tc.tile_pool(name="big", bufs=CT) as gpool,
        ):
            # beta_sb[p, a] = p_beta[a*128 + p]
            beta_sb = bpool.tile([P, CT], mybir.dt.float32)
            beta_eng = getattr(nc, BETA_ENGINE)
            beta_view = p_beta.ap().rearrange("(a p) -> p a", p=P)
            with nc.allow_non_contiguous_dma(
                reason="one-time 1KB load of p_beta, partition-strided"
            ):
                if BETA_SPLIT:
                    # One DMA per column: the ct0 fills gate only on the
                    # first (half-size) transfer.
                    for a in range(CT):
                        beta_eng.dma_start(
                            out=beta_sb[:, a : a + 1],
                            in_=beta_view[:, a : a + 1],
                        )
                else:
                    beta_eng.dma_start(out=beta_sb[:, :], in_=beta_view)

            def spans(early, rest):
                """`early` leading spans, then `rest`-sized spans up to N."""
                out, j = [], 0
                for w in early:
                    out.append((j, w))
                    j += w
                while j < N:
                    w = min(rest, N - j)
                    out.append((j, w))
                    j += w
                return out

            for ct in range(CT):
                big = gpool.tile([P, N], mybir.dt.float32)
                # Replicate the per-partition beta value across the free dim.
                # Small leading spans let the first output DMAs start early.
                early = EARLY_SPANS if ct == 0 else ()
                for j, w in spans(early, FILL_CHUNK):
                    nc.vector.tensor_copy(
                        out=big[:, j : j + w],
                        in_=beta_sb[:, ct : ct + 1].to_broadcast([P, w]),
                    )
                for it in range(IPC):
                    dma_early = EARLY_SPANS if (ct == 0 and it == 0) else ()
                    for di, (j, w) in enumerate(spans(dma_early, DMA_CHUNK)):
                        # Optionally issue the ramp DMAs alternately from the
                        # ACT HWDGE ring so descriptor generation overlaps.
                        eng = (nc.scalar if (RAMP_ALT_RING and ct == 0 and
                                             it == 0 and di % 2 == 0)
                               else nc.sync)
                        eng.dma_start(
                            out=out_ap[it, ct * P : (ct + 1) * P, j : j + w],
                            in_=big[:, j : j + w],
                        )
    nc.compile()
    return nc


def _get_nc():
    if "nc" not in _CACHE:
        _CACHE["nc"] = _build_nc()
    return _CACHE["nc"]


def kernel(**inputs) -> np.ndarray:
    global LAST_RESULTS
    p_beta = np.ascontiguousarray(np.asarray(inputs["p_beta"], dtype=np.float32))
    nc = _get_nc()
    in_maps = [{"p_beta": p_beta} for _ in range(N_CORES)]
    res = run_bass_kernel_spmd(nc, in_maps, core_ids=list(range(N_CORES)))
    LAST_RESULTS = res
    shards = [res.results[c]["out"] for c in range(N_CORES)]
    full = np.concatenate(shards, axis=0)          # [16, C, N]
    return full.reshape(T, B, C, Lt, Lh, Lw)

